# revision 39
# baseline (speedup 1.0000x reference)
"""Multi-head attention (B=2, S=2048, D=1024, H=16, causal) on 8 TRN2 NeuronCores.

Sharding: core c -> (batch b = c//4, head-group hg = c%4). Each core:
  - projects its batch's query/key/value against a 256-row slice of Wq/Wk/Wv
    (4 heads of 64 dims),
  - runs causal attention for those 4 heads (scores computed transposed,
    exp on ACT with fused 1/8 scale, row-sums via a ones-column in V),
  - multiplies by the matching 256-column slice of Wo -> partial [2048, 1024].
Host sums the 4 partials per batch (the tensor-parallel all-reduce) and stacks.

Everything runs in bf16 (fp32 PSUM accumulation): halves HBM traffic vs fp32
and keeps every matmul at 1 PE cycle/row. The emission builds ONE dense PE
stream: proj block 0 runs first, then the four attention blocks back-to-back
with the remaining projection / output-projection matmul groups interleaved
as fillers inside attention's slack slots, so the PE never idles and its
DVFS p-state stays at the fast clock.

Engine placement: exp on ACT (2 heads fused per instruction), PSUM->SBUF
copies on DVE, causal-mask + normalize multiplies on GpSimd (SBUF-only
engine, otherwise idle), softmax reciprocal via the fast custom-DVE approx.
"""

import sys

for _p in ("/opt/trn_rl_repo", "/root/.axon_site/_ro/trn_rl_repo"):
    if _p not in sys.path:
        sys.path.append(_p)

from collections import deque

import numpy as np
import ml_dtypes

import concourse.bacc as bacc
import concourse.tile as tile
import concourse.mybir as mybir
from concourse.bass import MemorySpace
from concourse.bass_utils import run_bass_kernel_spmd

f32 = mybir.dt.float32
f32r = mybir.dt.float32r
bf16 = mybir.dt.bfloat16
Exp = mybir.ActivationFunctionType.Exp
BF16 = ml_dtypes.bfloat16

B, S, D, H = 2, 2048, 1024, 16
HD = 64            # head dim
NH = 4             # heads per core
DO = NH * HD       # 256 projection out-dims per core
NCORES = 8
KI = D // 128      # 8 contraction chunks for the projections
QT = 512           # query block
NQT = S // QT      # 4
KT = 128           # key chunk
NT = QT // KT      # 4 key chunks per block
LEAD = 2           # exp/score lead over PV in the attention pipeline

_cache: dict = {}

# knobs for perf experiments
_opts = {"mask_engine": "dve", "norm_engine": "pool", "recip": "custom_f32r",
         "fuse_exp": True, "warmup": 6}


def _recip_fast(nc, out_ap, in_ap):
    """1/x at ~18 correct bits in one custom-DVE op (out may be f32r)."""
    from concourse.dve_ops import RECIP_APPROX_FAST_CONSTS, RECIPROCAL_APPROX_FAST

    c = RECIP_APPROX_FAST_CONSTS
    return nc.vector._custom_dve(
        RECIPROCAL_APPROX_FAST, out=out_ap, in0=in_ap,
        s0=c["s0"], s1=c["s1"], imm2=c["imm2"])


def _build():
    nc = bacc.Bacc("TRN2", target_bir_lowering=False, debug=False,
                   num_devices=NCORES)

    # host-prepacked layouts: x as [block, 128, KI*QT], weights as
    # [128, KI*DO] / [128, 2*D] so every DMA line is one contiguous run.
    xq_d = nc.dram_tensor("xq", [NQT, 128, KI * QT], bf16, kind="ExternalInput").ap()
    xk_d = nc.dram_tensor("xk", [NQT, 128, KI * QT], bf16, kind="ExternalInput").ap()
    xv_d = nc.dram_tensor("xv", [NQT, 128, KI * QT], bf16, kind="ExternalInput").ap()
    wq_d = nc.dram_tensor("wq", [128, KI * DO], bf16, kind="ExternalInput").ap()
    wk_d = nc.dram_tensor("wk", [128, KI * DO], bf16, kind="ExternalInput").ap()
    wv_d = nc.dram_tensor("wv", [128, KI * DO], bf16, kind="ExternalInput").ap()
    wo_d = nc.dram_tensor("wo", [128, 2 * D], bf16, kind="ExternalInput").ap()
    tri_d = nc.dram_tensor("tri", [128, 2 * KT], bf16, kind="ExternalInput").ap()
    out_d = nc.dram_tensor("out", [S, D], bf16, kind="ExternalOutput").ap()
    # kc=0 half-contraction partial of the last oproj block (host adds it):
    # lets those 8 matmuls run before the final pair's normalize lands
    out2_d = nc.dram_tensor("out2", [QT, D], bf16, kind="ExternalOutput").ap()

    with tile.TileContext(nc) as tc:
        with (
            tc.tile_pool(name="wpool", bufs=1) as wpool,
            tc.tile_pool(name="cpool", bufs=1) as cpool,
            tc.tile_pool(name="persist", bufs=1) as persist,
            tc.tile_pool(name="xq_p", bufs=2) as xq_p,
            tc.tile_pool(name="xk_p", bufs=2) as xk_p,
            tc.tile_pool(name="xv_p", bufs=2) as xv_p,
            tc.tile_pool(name="ptp", bufs=5) as ptp,
            tc.tile_pool(name="nsm", bufs=4) as nsm,
            tc.tile_pool(name="obuf", bufs=3) as obuf,
            tc.tile_pool(name="psS", bufs=2, space=MemorySpace.PSUM) as psS,
            tc.tile_pool(name="psA", bufs=2, space=MemorySpace.PSUM) as psA,
            tc.tile_pool(name="psO", bufs=2, space=MemorySpace.PSUM) as psO,
        ):
            _emit(nc, wpool, cpool, persist, xq_p, xk_p, xv_p, ptp, nsm,
                  obuf, psS, psA, psO, xq_d, xk_d, xv_d, wq_d, wk_d, wv_d,
                  wo_d, tri_d, out_d, out2_d)

    nc.compile()
    return nc


def _emit(nc, wpool, cpool, persist, xq_p, xk_p, xv_p, ptp, nsm, obuf,
          psS, psA, psO, xq_d, xk_d, xv_d, wq_d, wk_d, wv_d, wo_d, tri_d,
          out_d, out2_d):
    # ---- constants ----
    tri2 = cpool.tile([128, 2, KT], bf16, tag="tri", name="tri2")
    nc.sync.dma_start(tri2[:], tri_d.rearrange("p (a b) -> p a b", a=2))
    ones_f = cpool.tile([1, HD], f32, tag="ones_f", name="ones_f")
    nc.vector.memset(ones_f[:], 1.0)
    ones_sb = cpool.tile([1, HD], bf16, tag="ones", name="ones_sb")
    nc.vector.tensor_copy(ones_sb[:], ones_f[:])
    vones_f = cpool.tile([128, NT * NH], f32, tag="vones_f", name="vones_f")
    nc.vector.memset(vones_f[:], 1.0)
    wrm = cpool.tile([128, QT], f32, tag="wrm", name="wrm")
    nc.vector.memset(wrm[:], 0.001)

    # ---- weights: issue DMAs interleaved with x block 0 (priority order) ----
    wq_sb = wpool.tile([128, KI, DO], bf16, tag="wq", name="wq_sb")
    wk_sb = wpool.tile([128, KI, DO], bf16, tag="wk", name="wk_sb")
    wv_sb = wpool.tile([128, KI, DO], bf16, tag="wv", name="wv_sb")
    wo_sb = wpool.tile([128, 2, D], bf16, tag="wo", name="wo_sb")

    def load_x(t, which):
        pool, dram = {"q": (xq_p, xq_d), "k": (xk_p, xk_d),
                      "v": (xv_p, xv_d)}[which]
        xt = pool.tile([128, KI, QT], bf16, tag="x", name=f"x{which}")
        nc.sync.dma_start(xt[:], dram[t].rearrange("p (k n) -> p k n", k=KI))
        return xt

    nc.sync.dma_start(wq_sb[:], wq_d.rearrange("p (k n) -> p k n", k=KI))
    xq0 = load_x(0, "q")
    nc.sync.dma_start(wk_sb[:], wk_d.rearrange("p (k n) -> p k n", k=KI))
    xk0 = load_x(0, "k")
    nc.sync.dma_start(wv_sb[:], wv_d.rearrange("p (k n) -> p k n", k=KI))
    xv0 = load_x(0, "v")
    # wo is issued later (after block-1 x loads): not needed until oproj0

    # ---- persistent per-block tensors ----
    # qT/kT/oT: [128, 2, QT]; head j -> chunk j//2, partitions (j%2)*64..+64
    qTt = [persist.tile([128, 2, QT], bf16, tag=f"qT{t}", name=f"qT{t}")
           for t in range(NQT)]
    kTt = [persist.tile([128, 2, QT], bf16, tag=f"kT{t}", name=f"kT{t}")
           for t in range(NQT)]
    oTt = [persist.tile([128, 2, QT], bf16, tag=f"oT{t}", name=f"oT{t}")
           for t in range(NQT)]
    # v blocks, natural layout + ones column: [tok part, ktc, head, 65]
    vt = [persist.tile([128, NT, NH, HD + 1], bf16, tag=f"v{t}", name=f"v{t}")
          for t in range(NQT)]

    fill = deque()

    def pump():
        if fill:
            fill.popleft()()

    # ---- filler group builders ----
    def qk_group(d, xt, w_sb, dst):
        def g():
            ps = psA.tile([128, QT], f32, tag="ps", name="ps")
            for ki in range(KI):
                nc.tensor.matmul(ps[:], w_sb[:, ki, d * 128:(d + 1) * 128],
                                 xt[:, ki, :], start=(ki == 0),
                                 stop=(ki == KI - 1))
            nc.vector.tensor_copy(dst[:, d, :], ps[:])
        return g

    def v_ones(t):
        def g():
            nc.vector.tensor_copy(
                vt[t][:, :, :, HD],
                vones_f[:].rearrange("p (a b) -> p a b", a=NT))
        return g

    def v_group(t, tt, xt):
        def g():
            psv = psA.tile([128, DO], f32, tag="ps", name="psv")
            for ki in range(KI):
                nc.tensor.matmul(psv[:], xt[:, ki, tt * KT:(tt + 1) * KT],
                                 wv_sb[:, ki, :], start=(ki == 0),
                                 stop=(ki == KI - 1))
            nc.vector.tensor_copy(
                vt[t][:, tt, :, 0:HD],
                psv[:].rearrange("p (h e) -> p h e", h=NH))
        return g

    def proj_groups(t):
        xq = load_x(t, "q")
        xk = load_x(t, "k")
        xv = load_x(t, "v")
        gs = [qk_group(d, xq, wq_sb, qTt[t]) for d in range(2)]
        gs += [qk_group(d, xk, wk_sb, kTt[t]) for d in range(2)]
        gs.append(v_ones(t))
        gs += [v_group(t, tt, xv) for tt in range(NT)]
        return gs

    def oproj_groups(t):
        gs = []
        for mtt in range(NT):
            for n in range(D // QT):
                def g(mtt=mtt, n=n):
                    ps = psA.tile([128, QT], f32, tag="ps", name="pso2")
                    for kc in range(2):
                        nc.tensor.matmul(
                            ps[:], oTt[t][:, kc, mtt * KT:(mtt + 1) * KT],
                            wo_sb[:, kc, n * QT:(n + 1) * QT],
                            start=(kc == 0), stop=(kc == 1))
                    ob = obuf.tile([128, QT], bf16, tag="ob", name="ob")
                    nc.vector.tensor_copy(ob[:], ps[:])
                    mt = t * NT + mtt
                    nc.sync.dma_start(
                        out_d[mt * 128:(mt + 1) * 128, n * QT:(n + 1) * QT],
                        ob[:])
                gs.append(g)
        return gs

    def oproj_kc_groups(t, kc):
        # half-contraction oproj for block t: kc=0 -> out2 (host adds),
        # kc=1 -> out. Each half only needs ONE head pair's normalize.
        gs = []
        for mtt in range(NT):
            for n in range(D // QT):
                def g(mtt=mtt, n=n, kc=kc):
                    ps = psA.tile([128, QT], f32, tag="ps", name="pso2")
                    nc.tensor.matmul(
                        ps[:], oTt[t][:, kc, mtt * KT:(mtt + 1) * KT],
                        wo_sb[:, kc, n * QT:(n + 1) * QT],
                        start=True, stop=True)
                    ob = obuf.tile([128, QT], bf16, tag="ob", name="ob")
                    nc.vector.tensor_copy(ob[:], ps[:])
                    if kc == 0:
                        dst = out2_d[mtt * 128:(mtt + 1) * 128,
                                     n * QT:(n + 1) * QT]
                    else:
                        mt = t * NT + mtt
                        dst = out_d[mt * 128:(mt + 1) * 128,
                                    n * QT:(n + 1) * QT]
                    nc.sync.dma_start(dst, ob[:])
                gs.append(g)
        return gs

    # ---- attention ----
    # The whole block is one flat stream of (pair, kt) items: scores run
    # LEAD items ahead of PV continuously, ACROSS pair boundaries, so the
    # exp pipeline never drains. Items are emitted in batches of two
    # (S,S,PV,PV) to cut PE weight-tile reconfigs. Normalize work is
    # scheduled onto future batches via a global batch counter so its
    # broadcast matmuls never wait on the just-issued DVE reciprocal.
    gb = [0]      # global batch counter
    sched = []    # (due_batch, closure) heap-ish list

    def sched_at(delay, fn):
        sched.append((gb[0] + delay, fn))

    def run_due():
        for e in sorted([e for e in sched if e[0] <= gb[0]], key=lambda x: x[0]):
            sched.remove(e)
            e[1]()

    def flush_norms():
        for e in sorted(sched, key=lambda x: x[0]):
            e[1]()
        sched.clear()

    def attn_block(qt, extra=(), late_extra=()):
        nkt = (qt + 1) * NT
        items = [(d, kt) for d in (0, 1) for kt in range(nkt)]
        n = len(items)
        nbatch = n // 2 + 1
        per = (len(fill) + len(extra) + len(late_extra)) / nbatch
        acc = 0.0
        pso_of = {}
        window = {}

        if extra:
            # oproj fillers for block qt-1 join once its last norm (sched'd
            # into this block's early batches) has been emitted
            sched_at(5, lambda e=extra: fill.extend(e))
        if late_extra:
            # work that depends on THIS block's first-pair normalize
            # (emitted ~batch n//4+4): join after it
            sched_at(n // 4 + 5, lambda e=late_extra: fill.extend(e))

        def emit_S(item):
            d, kt = item
            r = kt - qt * NT
            co = max(r, 0) * KT
            w = QT - co
            pss = psS.tile([128, 2, QT], f32, tag="pss", name="pss")
            for jj in (0, 1):
                kh = kTt[kt // NT][jj * HD:(jj + 1) * HD, d,
                                   (kt % NT) * KT:(kt % NT + 1) * KT]
                nc.tensor.matmul(pss[:, jj, 0:w], kh,
                                 qTt[qt][jj * HD:(jj + 1) * HD, d, co:QT],
                                 start=True, stop=True)
            pt = ptp.tile([128, 2, QT], bf16, tag="pt", name="pt")
            nc.scalar.activation(pt[:, :, 0:w], pss[:, :, 0:w], Exp,
                                 scale=0.125)
            if r >= 0:
                if _opts["mask_engine"] == "pool":
                    nc.gpsimd.tensor_mul(pt[:, :, 0:KT], pt[:, :, 0:KT],
                                         tri2[:])
                else:
                    nc.vector.tensor_mul(pt[:, :, 0:KT], pt[:, :, 0:KT],
                                         tri2[:])
            window[item] = (co, w, pt)

        def emit_PV(item):
            d, kt = item
            if d not in pso_of:
                pso_of[d] = [psO.tile([HD + 1, QT], f32, tag="pso",
                                      name=f"pso{jj}") for jj in (0, 1)]
            pso = pso_of[d]
            co, w, pt = window.pop(item)
            for jj in (0, 1):
                nc.tensor.matmul(
                    pso[jj][:, co:QT],
                    vt[kt // NT][:, kt % NT, 2 * d + jj, :],
                    pt[:, jj, 0:w],
                    start=(kt == 0), stop=(kt == nkt - 1))
            if kt == nkt - 1:
                # pair finished: stage A next batch (frees the psO ring
                # before the next pair's PVs), stage B a few batches later
                def stageA(d=d, pso=pso):
                    fnB = _norm_stageA(qt, d, pso)
                    sched_at(3, fnB)
                sched_at(1, stageA)

        for i in range(0, n + LEAD + 1, 2):
            run_due()
            for k in (i, i + 1):
                if k < n:
                    emit_S(items[k])
            for k in (i - LEAD, i - LEAD + 1):
                if 0 <= k < n:
                    emit_PV(items[k])
            gb[0] += 1
            # fillers go AFTER the batch's own work so they never
            # head-of-line block the attention stream in the PE queue
            acc += per
            while acc >= 1.0:
                pump()
                acc -= 1.0

    def _norm_stageA(qt, d, pso):
        parts = []
        for jj in (0, 1):
            oun = nsm.tile([HD, QT], f32, tag="oun", name="oun")
            nc.vector.tensor_copy(oun[:], pso[jj][0:HD, :])
            # rowsum row to an offset-0 tile (custom-DVE ops need offset-0)
            rs0 = nsm.tile([1, QT], f32, tag="rs0", name="rs0")
            nc.vector.tensor_copy(rs0[:], pso[jj][HD:HD + 1, :])
            recir = nsm.tile([1, QT], bf16, tag="recir", name="recir")
            mode = _opts["recip"]
            with nc.allow_low_precision(reason="f32r normalization scale"):
                if mode == "custom_f32r":
                    _recip_fast(nc, recir[:], rs0[:])
                elif mode == "lnexp":
                    # 1/x = exp(-ln x); Ln+Exp share one ACT table set
                    rf = nsm.tile([1, QT], f32, tag="rf", name="rf")
                    nc.scalar.activation(rf[:], rs0[:],
                                         mybir.ActivationFunctionType.Ln)
                    nc.scalar.activation(recir[:], rf[:], Exp, scale=-1.0)
                else:
                    nc.vector.reciprocal(recir[:], rs0[:])
            parts.append((jj, oun, recir))
        return lambda: _norm_stageB(qt, d, parts)

    def _norm_stageB(qt, d, parts):
        for jj, oun, recir in parts:
            psb = psA.tile([HD, QT], f32, tag="ps", name="psb")
            nc.tensor.matmul(psb[:], ones_sb[:], recir[:],
                             start=True, stop=True)
            bc = nsm.tile([HD, QT], f32, tag="bc", name="bc")
            nc.vector.tensor_copy(bc[:], psb[:])
            dst = oTt[qt][jj * HD:(jj + 1) * HD, d, :]
            if _opts["norm_engine"] == "pool":
                nc.gpsimd.tensor_mul(dst, oun[:], bc[:])
            else:
                nc.vector.tensor_mul(dst, oun[:], bc[:])

    # ---- schedule ----
    # warm-up matmuls on a const tile while the first DMAs stream in: ramps
    # the PE DVFS clock so proj0 starts at full speed (results unused)
    for _ in range(_opts["warmup"]):
        pw = psA.tile([128, QT], f32, tag="ps", name="pw")
        nc.tensor.matmul(pw[:], wrm[:, 0:128], wrm[:], start=True, stop=True)

    # proj0 runs directly; everything else rides the filler queue inside the
    # attention blocks. oproj groups of block t are handed to attn block t+1
    # as `extra` so they can't be emitted before their oTt norms.
    for g in ([qk_group(d, xq0, wq_sb, qTt[0]) for d in range(2)]
              + [qk_group(d, xk0, wk_sb, kTt[0]) for d in range(2)]
              + [v_ones(0)]
              + [v_group(0, tt, xv0) for tt in range(NT)]):
        g()

    fill.extend(proj_groups(1))
    nc.sync.dma_start(wo_sb[:], wo_d.rearrange("p (k n) -> p k n", k=2))
    attn_block(0)
    fill.extend(proj_groups(2))
    attn_block(1, extra=oproj_groups(0))
    fill.extend(proj_groups(3))
    attn_block(2, extra=oproj_groups(1))
    # block 3's oproj is kc-split: the kc=0 half (head pair 0) runs inside
    # the block once pair 0 is normalized; kc=1 follows the final normalize
    attn_block(3, extra=oproj_groups(2), late_extra=oproj_kc_groups(3, 0))
    # trailing batches: drain fillers while the final pair's normalize
    # (scheduled on the batch counter) lands
    for _ in range(3):
        run_due()
        gb[0] += 1
        pump()
        pump()
    flush_norms()
    fill.extend(oproj_kc_groups(3, 1))
    while fill:
        pump()


def _pack_x(x):
    """[S, D] fp32 (one batch) -> [NQT, 128, KI*QT] bf16, transposed layout."""
    xT = np.ascontiguousarray(x.T).astype(BF16)  # [D, S]
    return np.ascontiguousarray(
        xT.reshape(KI, 128, NQT, QT).transpose(2, 1, 0, 3)
        .reshape(NQT, 128, KI * QT))


def _pack_w(wT, chunks):
    """[din, dout] fp32 (pre-transposed) -> [128, chunks*dout] bf16."""
    dout = wT.shape[1]
    return np.ascontiguousarray(
        wT.astype(BF16).reshape(chunks, 128, dout).transpose(1, 0, 2)
        .reshape(128, chunks * dout))


def _mask_tiles():
    i = np.arange(128)[:, None]
    j = np.arange(KT)[None, :]
    tri = (j >= i).astype(np.float32)
    return np.concatenate([tri, tri], axis=1).astype(BF16)


def make_in_maps(query, key, value, Wq, Wk, Wv, Wo):
    query = np.asarray(query, np.float32)
    key = np.asarray(key, np.float32)
    value = np.asarray(value, np.float32)
    Wq = np.asarray(Wq, np.float32)
    Wk = np.asarray(Wk, np.float32)
    Wv = np.asarray(Wv, np.float32)
    Wo = np.asarray(Wo, np.float32)
    tri = _mask_tiles()
    xq_b = [_pack_x(query[b]) for b in range(B)]
    xk_b = [_pack_x(key[b]) for b in range(B)]
    xv_b = [_pack_x(value[b]) for b in range(B)]
    in_maps = []
    for c in range(NCORES):
        b, hg = divmod(c, NCORES // B)
        sl = slice(hg * DO, (hg + 1) * DO)
        in_maps.append({
            "xq": xq_b[b],
            "xk": xk_b[b],
            "xv": xv_b[b],
            "wq": _pack_w(np.ascontiguousarray(Wq[sl].T), KI),
            "wk": _pack_w(np.ascontiguousarray(Wk[sl].T), KI),
            "wv": _pack_w(np.ascontiguousarray(Wv[sl].T), KI),
            "wo": _pack_w(np.ascontiguousarray(Wo[:, sl].T), 2),
            "tri": tri,
        })
    return in_maps


def kernel(query, key, value, freqs_complex_form, mask, Wq, Wk, Wv, Wo):
    if "nc" not in _cache:
        _cache["nc"] = _build()
    nc = _cache["nc"]
    in_maps = make_in_maps(query, key, value, Wq, Wk, Wv, Wo)
    res = run_bass_kernel_spmd(nc, in_maps, list(range(NCORES)))
    parts = []
    for c in range(NCORES):
        p = np.asarray(res.results[c]["out"]).astype(np.float32)
        p[S - QT:S] += np.asarray(res.results[c]["out2"]).astype(np.float32)
        parts.append(p)
    npg = NCORES // B
    return np.stack(
        [np.sum(parts[b * npg:(b + 1) * npg], axis=0) for b in range(B)]
    ).astype(np.float32)


# revision 40
# speedup vs baseline: 1.0291x; 1.0291x over previous
"""Multi-head attention (B=2, S=2048, D=1024, H=16, causal) on 8 TRN2 NeuronCores.

Sharding: core c -> (batch b = c//4, head-group hg = c%4). Each core:
  - projects its batch's query/key/value against a 256-row slice of Wq/Wk/Wv
    (4 heads of 64 dims),
  - runs causal attention for those 4 heads (scores computed transposed,
    exp on ACT with fused 1/8 scale, row-sums via a ones-column in V),
  - multiplies by the matching 256-column slice of Wo -> partial [2048, 1024].
Host sums the 4 partials per batch (the tensor-parallel all-reduce) and stacks.

Everything runs in bf16 (fp32 PSUM accumulation): halves HBM traffic vs fp32
and keeps every matmul at 1 PE cycle/row. The emission builds ONE dense PE
stream: proj block 0 runs first, then the four attention blocks back-to-back
with the remaining projection / output-projection matmul groups interleaved
as fillers inside attention's slack slots, so the PE never idles and its
DVFS p-state stays at the fast clock.

Engine placement: exp on ACT (2 heads fused per instruction), PSUM->SBUF
copies on DVE, causal-mask + normalize multiplies on GpSimd (SBUF-only
engine, otherwise idle), softmax reciprocal via the fast custom-DVE approx.
"""

import sys

for _p in ("/opt/trn_rl_repo", "/root/.axon_site/_ro/trn_rl_repo"):
    if _p not in sys.path:
        sys.path.append(_p)

from collections import deque

import numpy as np
import ml_dtypes

import concourse.bacc as bacc
import concourse.tile as tile
import concourse.mybir as mybir
from concourse.bass import MemorySpace
from concourse.bass_utils import run_bass_kernel_spmd

f32 = mybir.dt.float32
f32r = mybir.dt.float32r
bf16 = mybir.dt.bfloat16
Exp = mybir.ActivationFunctionType.Exp
BF16 = ml_dtypes.bfloat16

B, S, D, H = 2, 2048, 1024, 16
HD = 64            # head dim
NH = 4             # heads per core
DO = NH * HD       # 256 projection out-dims per core
NCORES = 8
KI = D // 128      # 8 contraction chunks for the projections
QT = 512           # query block
NQT = S // QT      # 4
KT = 128           # key chunk
NT = QT // KT      # 4 key chunks per block
LEAD = 2           # exp/score lead over PV in the attention pipeline

_cache: dict = {}

# knobs for perf experiments
_opts = {"mask_engine": "dve", "norm_engine": "pool", "recip": "custom_f32r",
         "fuse_exp": True, "warmup": 6}


def _recip_fast(nc, out_ap, in_ap):
    """1/x at ~18 correct bits in one custom-DVE op (out may be f32r)."""
    from concourse.dve_ops import RECIP_APPROX_FAST_CONSTS, RECIPROCAL_APPROX_FAST

    c = RECIP_APPROX_FAST_CONSTS
    return nc.vector._custom_dve(
        RECIPROCAL_APPROX_FAST, out=out_ap, in0=in_ap,
        s0=c["s0"], s1=c["s1"], imm2=c["imm2"])


def _build():
    nc = bacc.Bacc("TRN2", target_bir_lowering=False, debug=False,
                   num_devices=NCORES)

    # host-prepacked layouts: x as [block, 128, KI*QT], weights as
    # [128, KI*DO] / [128, 2*D] so every DMA line is one contiguous run.
    xq_d = nc.dram_tensor("xq", [NQT, 128, KI * QT], bf16, kind="ExternalInput").ap()
    xk_d = nc.dram_tensor("xk", [NQT, 128, KI * QT], bf16, kind="ExternalInput").ap()
    xv_d = nc.dram_tensor("xv", [NQT, 128, KI * QT], bf16, kind="ExternalInput").ap()
    wq_d = nc.dram_tensor("wq", [128, KI * DO], bf16, kind="ExternalInput").ap()
    wk_d = nc.dram_tensor("wk", [128, KI * DO], bf16, kind="ExternalInput").ap()
    wv_d = nc.dram_tensor("wv", [128, KI * DO], bf16, kind="ExternalInput").ap()
    wo_d = nc.dram_tensor("wo", [128, 2 * D], bf16, kind="ExternalInput").ap()
    tri_d = nc.dram_tensor("tri", [128, 2 * KT], bf16, kind="ExternalInput").ap()
    out_d = nc.dram_tensor("out", [S, D], bf16, kind="ExternalOutput").ap()

    with tile.TileContext(nc) as tc:
        with (
            tc.tile_pool(name="wpool", bufs=1) as wpool,
            tc.tile_pool(name="cpool", bufs=1) as cpool,
            tc.tile_pool(name="persist", bufs=1) as persist,
            tc.tile_pool(name="xq_p", bufs=2) as xq_p,
            tc.tile_pool(name="xk_p", bufs=2) as xk_p,
            tc.tile_pool(name="xv_p", bufs=2) as xv_p,
            tc.tile_pool(name="ptp", bufs=4) as ptp,
            tc.tile_pool(name="nsm", bufs=4) as nsm,
            tc.tile_pool(name="obuf", bufs=3) as obuf,
            tc.tile_pool(name="psS", bufs=2, space=MemorySpace.PSUM) as psS,
            tc.tile_pool(name="psA", bufs=2, space=MemorySpace.PSUM) as psA,
            tc.tile_pool(name="psO", bufs=2, space=MemorySpace.PSUM) as psO,
        ):
            _emit(nc, wpool, cpool, persist, xq_p, xk_p, xv_p, ptp, nsm,
                  obuf, psS, psA, psO, xq_d, xk_d, xv_d, wq_d, wk_d, wv_d,
                  wo_d, tri_d, out_d)

    nc.compile()
    return nc


def _emit(nc, wpool, cpool, persist, xq_p, xk_p, xv_p, ptp, nsm, obuf,
          psS, psA, psO, xq_d, xk_d, xv_d, wq_d, wk_d, wv_d, wo_d, tri_d,
          out_d):
    # ---- constants ----
    tri2 = cpool.tile([128, 2, KT], bf16, tag="tri", name="tri2")
    nc.sync.dma_start(tri2[:], tri_d.rearrange("p (a b) -> p a b", a=2))
    ones_f = cpool.tile([1, HD], f32, tag="ones_f", name="ones_f")
    nc.vector.memset(ones_f[:], 1.0)
    ones_sb = cpool.tile([1, HD], f32r, tag="ones", name="ones_sb")
    nc.vector.tensor_copy(ones_sb[:], ones_f[:])
    vones_f = cpool.tile([128, NT * NH], f32, tag="vones_f", name="vones_f")
    nc.vector.memset(vones_f[:], 1.0)
    wrm = cpool.tile([128, QT], f32, tag="wrm", name="wrm")
    nc.vector.memset(wrm[:], 0.001)

    # ---- weights: issue DMAs interleaved with x block 0 (priority order) ----
    wq_sb = wpool.tile([128, KI, DO], bf16, tag="wq", name="wq_sb")
    wk_sb = wpool.tile([128, KI, DO], bf16, tag="wk", name="wk_sb")
    wv_sb = wpool.tile([128, KI, DO], bf16, tag="wv", name="wv_sb")
    wo_sb = wpool.tile([128, 2, D], bf16, tag="wo", name="wo_sb")

    def load_x(t, which):
        pool, dram = {"q": (xq_p, xq_d), "k": (xk_p, xk_d),
                      "v": (xv_p, xv_d)}[which]
        xt = pool.tile([128, KI, QT], bf16, tag="x", name=f"x{which}")
        nc.sync.dma_start(xt[:], dram[t].rearrange("p (k n) -> p k n", k=KI))
        return xt

    nc.sync.dma_start(wq_sb[:], wq_d.rearrange("p (k n) -> p k n", k=KI))
    xq0 = load_x(0, "q")
    nc.sync.dma_start(wk_sb[:], wk_d.rearrange("p (k n) -> p k n", k=KI))
    xk0 = load_x(0, "k")
    nc.sync.dma_start(wv_sb[:], wv_d.rearrange("p (k n) -> p k n", k=KI))
    xv0 = load_x(0, "v")
    # wo is issued later (after block-1 x loads): not needed until oproj0

    # ---- persistent per-block tensors ----
    # qT/kT/oT: [128, 2, QT]; head j -> chunk j//2, partitions (j%2)*64..+64
    qTt = [persist.tile([128, 2, QT], bf16, tag=f"qT{t}", name=f"qT{t}")
           for t in range(NQT)]
    kTt = [persist.tile([128, 2, QT], bf16, tag=f"kT{t}", name=f"kT{t}")
           for t in range(NQT)]
    oTt = [persist.tile([128, 2, QT], bf16, tag=f"oT{t}", name=f"oT{t}")
           for t in range(NQT)]
    # v blocks, natural layout + ones column: [tok part, ktc, head, 65]
    vt = [persist.tile([128, NT, NH, HD + 1], bf16, tag=f"v{t}", name=f"v{t}")
          for t in range(NQT)]

    fill = deque()

    def pump():
        if fill:
            fill.popleft()()

    # ---- filler group builders ----
    def qk_group(d, xt, w_sb, dst):
        def g():
            ps = psA.tile([128, QT], f32, tag="ps", name="ps")
            for ki in range(KI):
                nc.tensor.matmul(ps[:], w_sb[:, ki, d * 128:(d + 1) * 128],
                                 xt[:, ki, :], start=(ki == 0),
                                 stop=(ki == KI - 1))
            nc.vector.tensor_copy(dst[:, d, :], ps[:])
        return g

    def v_ones(t):
        def g():
            nc.vector.tensor_copy(
                vt[t][:, :, :, HD],
                vones_f[:].rearrange("p (a b) -> p a b", a=NT))
        return g

    def v_group(t, tt, xt):
        def g():
            psv = psA.tile([128, DO], f32, tag="ps", name="psv")
            for ki in range(KI):
                nc.tensor.matmul(psv[:], xt[:, ki, tt * KT:(tt + 1) * KT],
                                 wv_sb[:, ki, :], start=(ki == 0),
                                 stop=(ki == KI - 1))
            nc.vector.tensor_copy(
                vt[t][:, tt, :, 0:HD],
                psv[:].rearrange("p (h e) -> p h e", h=NH))
        return g

    def proj_groups(t):
        xq = load_x(t, "q")
        xk = load_x(t, "k")
        xv = load_x(t, "v")
        gs = [qk_group(d, xq, wq_sb, qTt[t]) for d in range(2)]
        gs += [qk_group(d, xk, wk_sb, kTt[t]) for d in range(2)]
        gs.append(v_ones(t))
        gs += [v_group(t, tt, xv) for tt in range(NT)]
        return gs

    def oproj_groups(t):
        gs = []
        for mtt in range(NT):
            for n in range(D // QT):
                def g(mtt=mtt, n=n):
                    ps = psA.tile([128, QT], f32, tag="ps", name="pso2")
                    for kc in range(2):
                        nc.tensor.matmul(
                            ps[:], oTt[t][:, kc, mtt * KT:(mtt + 1) * KT],
                            wo_sb[:, kc, n * QT:(n + 1) * QT],
                            start=(kc == 0), stop=(kc == 1))
                    ob = obuf.tile([128, QT], bf16, tag="ob", name="ob")
                    nc.vector.tensor_copy(ob[:], ps[:])
                    mt = t * NT + mtt
                    nc.sync.dma_start(
                        out_d[mt * 128:(mt + 1) * 128, n * QT:(n + 1) * QT],
                        ob[:])
                gs.append(g)
        return gs

    # ---- attention ----
    # normalize emission is deferred into the next pair's steps: stage A
    # (reciprocal chain, DVE-only) at step 2, stage B (broadcast matmul +
    # final multiply) at step 5 — so the broadcast matmul never sits in the
    # PE queue waiting on the just-issued DVE reciprocal
    pending_A = []
    pending_B = []

    def flush_norms():
        while pending_A:
            pending_B.append(pending_A.pop(0)())
        while pending_B:
            pending_B.pop(0)()

    def attn_block(qt, extra=()):
        # `extra` filler groups may read oTt written by still-pending norms,
        # so they join the queue only after the step-2 flush emits those norms
        nkt = (qt + 1) * NT
        slots = 2 * (nkt + LEAD)
        per = (len(fill) + len(extra)) / slots
        acc = 0.0

        for d in (0, 1):  # head pair (2d, 2d+1)
            window = {}
            pso = [psO.tile([HD + 1, QT], f32, tag="pso", name=f"pso{jj}")
                   for jj in (0, 1)]
            for step in range(nkt + LEAD):
                if step == 2:
                    while pending_A:
                        pending_B.append(pending_A.pop(0)())
                if step == 5:
                    while pending_B:
                        pending_B.pop(0)()
                    if d == 0 and extra:
                        fill.extend(extra)
                        extra = ()
                if step < nkt:
                    kt = step
                    r = kt - qt * NT
                    co = max(r, 0) * KT
                    w = QT - co
                    pss = psS.tile([128, 2, QT], f32, tag="pss", name="pss")
                    for jj in (0, 1):
                        kh = kTt[kt // NT][jj * HD:(jj + 1) * HD, d,
                                           (kt % NT) * KT:(kt % NT + 1) * KT]
                        nc.tensor.matmul(pss[:, jj, 0:w], kh,
                                         qTt[qt][jj * HD:(jj + 1) * HD, d, co:QT],
                                         start=True, stop=True)
                    pt = ptp.tile([128, 2, QT], bf16, tag="pt", name="pt")
                    if _opts["fuse_exp"]:
                        nc.scalar.activation(pt[:, :, 0:w], pss[:, :, 0:w], Exp,
                                             scale=0.125)
                    else:
                        for jj in (0, 1):
                            nc.scalar.activation(pt[:, jj, 0:w], pss[:, jj, 0:w],
                                                 Exp, scale=0.125)
                    if r >= 0:
                        if _opts["mask_engine"] == "pool":
                            nc.gpsimd.tensor_mul(pt[:, :, 0:KT], pt[:, :, 0:KT],
                                                 tri2[:])
                        else:
                            nc.vector.tensor_mul(pt[:, :, 0:KT], pt[:, :, 0:KT],
                                                 tri2[:])
                    window[kt] = (co, w, pt)
                if step >= LEAD:
                    kt = step - LEAD
                    co, w, pt = window.pop(kt)
                    for jj in (0, 1):
                        nc.tensor.matmul(
                            pso[jj][:, co:QT],
                            vt[kt // NT][:, kt % NT, 2 * d + jj, :],
                            pt[:, jj, 0:w],
                            start=(kt == 0), stop=(kt == nkt - 1))
                # fillers go AFTER the step's own work so they never
                # head-of-line block the attention stream in the PE queue
                acc += per
                while acc >= 1.0:
                    pump()
                    acc -= 1.0
            # normalize the pair: columns scaled by 1/rowsum (deferred)
            pending_A.append(lambda d=d, pso=pso: _norm_stageA(qt, d, pso))

    def _norm_stageA(qt, d, pso):
        parts = []
        for jj in (0, 1):
            oun = nsm.tile([HD, QT], f32, tag="oun", name="oun")
            nc.vector.tensor_copy(oun[:], pso[jj][0:HD, :])
            # rowsum row to an offset-0 tile (custom-DVE ops need offset-0)
            rs0 = nsm.tile([1, QT], f32, tag="rs0", name="rs0")
            nc.vector.tensor_copy(rs0[:], pso[jj][HD:HD + 1, :])
            recir = nsm.tile([1, QT], f32r, tag="recir", name="recir")
            mode = _opts["recip"]
            with nc.allow_low_precision(reason="f32r normalization scale"):
                if mode == "custom_f32r":
                    _recip_fast(nc, recir[:], rs0[:])
                elif mode == "lnexp":
                    # 1/x = exp(-ln x); Ln+Exp share one ACT table set
                    rf = nsm.tile([1, QT], f32, tag="rf", name="rf")
                    nc.scalar.activation(rf[:], rs0[:],
                                         mybir.ActivationFunctionType.Ln)
                    nc.scalar.activation(recir[:], rf[:], Exp, scale=-1.0)
                else:
                    nc.vector.reciprocal(recir[:], rs0[:])
            parts.append((jj, oun, recir))
        return lambda: _norm_stageB(qt, d, parts)

    def _norm_stageB(qt, d, parts):
        for jj, oun, recir in parts:
            psb = psA.tile([HD, QT], f32, tag="ps", name="psb")
            nc.tensor.matmul(psb[:], ones_sb[:], recir[:],
                             start=True, stop=True)
            bc = nsm.tile([HD, QT], f32, tag="bc", name="bc")
            nc.vector.tensor_copy(bc[:], psb[:])
            dst = oTt[qt][jj * HD:(jj + 1) * HD, d, :]
            if _opts["norm_engine"] == "pool":
                nc.gpsimd.tensor_mul(dst, oun[:], bc[:])
            else:
                nc.vector.tensor_mul(dst, oun[:], bc[:])

    # ---- schedule ----
    # warm-up matmuls on a const tile while the first DMAs stream in: ramps
    # the PE DVFS clock so proj0 starts at full speed (results unused)
    for _ in range(_opts["warmup"]):
        pw = psA.tile([128, QT], f32, tag="ps", name="pw")
        nc.tensor.matmul(pw[:], wrm[:, 0:128], wrm[:], start=True, stop=True)

    # proj0 runs directly; everything else rides the filler queue inside the
    # attention blocks. oproj groups of block t are handed to attn block t+1
    # as `extra` so they can't be emitted before their oTt norms.
    for g in ([qk_group(d, xq0, wq_sb, qTt[0]) for d in range(2)]
              + [qk_group(d, xk0, wk_sb, kTt[0]) for d in range(2)]
              + [v_ones(0)]
              + [v_group(0, tt, xv0) for tt in range(NT)]):
        g()

    fill.extend(proj_groups(1))
    nc.sync.dma_start(wo_sb[:], wo_d.rearrange("p (k n) -> p k n", k=2))
    attn_block(0)
    fill.extend(proj_groups(2))
    attn_block(1, extra=oproj_groups(0))
    fill.extend(proj_groups(3))
    attn_block(2, extra=oproj_groups(1))
    attn_block(3, extra=oproj_groups(2))
    flush_norms()
    fill.extend(oproj_groups(3))
    while fill:
        pump()


def _pack_x(x):
    """[S, D] fp32 (one batch) -> [NQT, 128, KI*QT] bf16, transposed layout."""
    xT = np.ascontiguousarray(x.T).astype(BF16)  # [D, S]
    return np.ascontiguousarray(
        xT.reshape(KI, 128, NQT, QT).transpose(2, 1, 0, 3)
        .reshape(NQT, 128, KI * QT))


def _pack_w(wT, chunks):
    """[din, dout] fp32 (pre-transposed) -> [128, chunks*dout] bf16."""
    dout = wT.shape[1]
    return np.ascontiguousarray(
        wT.astype(BF16).reshape(chunks, 128, dout).transpose(1, 0, 2)
        .reshape(128, chunks * dout))


def _mask_tiles():
    i = np.arange(128)[:, None]
    j = np.arange(KT)[None, :]
    tri = (j >= i).astype(np.float32)
    return np.concatenate([tri, tri], axis=1).astype(BF16)


def make_in_maps(query, key, value, Wq, Wk, Wv, Wo):
    query = np.asarray(query, np.float32)
    key = np.asarray(key, np.float32)
    value = np.asarray(value, np.float32)
    Wq = np.asarray(Wq, np.float32)
    Wk = np.asarray(Wk, np.float32)
    Wv = np.asarray(Wv, np.float32)
    Wo = np.asarray(Wo, np.float32)
    tri = _mask_tiles()
    xq_b = [_pack_x(query[b]) for b in range(B)]
    xk_b = [_pack_x(key[b]) for b in range(B)]
    xv_b = [_pack_x(value[b]) for b in range(B)]
    in_maps = []
    for c in range(NCORES):
        b, hg = divmod(c, NCORES // B)
        sl = slice(hg * DO, (hg + 1) * DO)
        in_maps.append({
            "xq": xq_b[b],
            "xk": xk_b[b],
            "xv": xv_b[b],
            "wq": _pack_w(np.ascontiguousarray(Wq[sl].T), KI),
            "wk": _pack_w(np.ascontiguousarray(Wk[sl].T), KI),
            "wv": _pack_w(np.ascontiguousarray(Wv[sl].T), KI),
            "wo": _pack_w(np.ascontiguousarray(Wo[:, sl].T), 2),
            "tri": tri,
        })
    return in_maps


def kernel(query, key, value, freqs_complex_form, mask, Wq, Wk, Wv, Wo):
    if "nc" not in _cache:
        _cache["nc"] = _build()
    nc = _cache["nc"]
    in_maps = make_in_maps(query, key, value, Wq, Wk, Wv, Wo)
    res = run_bass_kernel_spmd(nc, in_maps, list(range(NCORES)))
    parts = [np.asarray(res.results[c]["out"]).astype(np.float32)
             for c in range(NCORES)]
    npg = NCORES // B
    return np.stack(
        [np.sum(parts[b * npg:(b + 1) * npg], axis=0) for b in range(B)]
    ).astype(np.float32)
